# revision 37
# baseline (speedup 1.0000x reference)
"""Trainium2 Bass kernel: bilinear interpolation from BEV feature maps.

reference semantics (interpolate_from_bev_features, correction=False):
  keypoints (B, N, 3) f32; bev_features (B, C, H, W) f32; bev_stride scalar
  out (B, N, C) f32: bilinear sample at x = kp_x/(0.05*stride),
  y = (kp_y+40)/(0.05*stride); corner indices clamped to [0, 187]; weights
  from clamped corner coords (out-of-range y cancels to exactly 0).

Sharding: 8 cores = batch (4) x channel-half (2).

Per-core plan (SBUF gather ucode measured ~20 ns/element on this part, so
the gather runs on the DMA engines instead):
  Phase A: stream-transpose the (128ch, H*W_PACK) slab into a DRAM scratch
    TBEV[px, 128ch]: DMA load [128, 2048] -> DVE 32x32 stream-transpose ->
    4 DMA stores with block-permuted 3D access patterns (128B runs).
  Phase B: dma_gather (MoE-style SWDGE gather) fetches, per keypoint corner
    row, a 384-element run (3 pixels x 128ch starting at the even pixel
    below x0) out of an overlapped [V, 384]/stride-256 view of TBEV.
    int16 gather indices address 256-element pair rows (max 16731).
    The x-parity selects which 2 of the 3 pixels matter - folded into
    per-keypoint 3-slot weights, applied on DVE via stride-0 broadcast.
    Output lands keypoint-major: straight DMA out.

Shapes hardcoded per problem spec: B=4 N=4096 C=256 H=W=188 (x<=176 so
only W_PACK=178 columns are ever addressed).
"""
import os
import sys

for _p in ('/opt/trn_rl_repo', '/root/.axon_site/_ro/trn_rl_repo'):
    if os.path.isdir(_p) and _p not in sys.path:
        sys.path.append(_p)

import numpy as np

B, N, C, H, W = 4, 4096, 256, 188, 188
W_PACK = 178                  # x <= 176 -> x1 <= 177; cols 178..187 never read
FLAT = H * W_PACK             # 33464 pixels
NLOAD = 17                    # phase-A loads of [128, 2048]
FLAT_PAD = NLOAD * 2048       # 34816
VPAIR = FLAT_PAD * 128 // 256 - 1   # overlapped 384-elem rows, stride 256
BCH = 512                     # gather indices per dma_gather call
NBCH = N // BCH               # 4 phase-B chunks
GPC = BCH // 128              # keypoint blocks of 128 per chunk (8)
N_CORES = 8

_compiled = {}


def _build(scale: float, ybias: float, debug_taps: bool = False):
    import concourse.bacc as bacc
    import concourse.mybir as mybir
    import concourse.tile as tile
    import contextlib
    from concourse.bass import AP

    dt = mybir.dt
    nc = bacc.Bacc("TRN2", target_bir_lowering=False, debug=False,
                   num_devices=N_CORES)

    slab_d = nc.dram_tensor("slab", [128, FLAT_PAD], dt.float32, kind="ExternalInput")
    kp_d = nc.dram_tensor("kp", [N, 3], dt.float32, kind="ExternalInput")
    out_d = nc.dram_tensor("out", [N, 128], dt.float32, kind="ExternalOutput")
    tbev_d = nc.dram_tensor("tbev", [FLAT_PAD, 128], dt.float32)

    taps = {}
    if debug_taps:
        for nm, shp in [("t_X0", [128, 32]), ("t_QX", [128, 32]),
                        ("t_W3AC", [128, 96]), ("t_I0w", [128, 256]),
                        ("t_TB", [128, 128])]:
            taps[nm] = nc.dram_tensor(nm, shp, dt.float32, kind="ExternalOutput")

    # keypoint n = s*128 + p  (block-major) for weights;
    # n = s*16 + r (wrapped-16) for gather indices
    kp_blk = kp_d.ap().rearrange("(s p) c -> p s c", p=128)        # [128, 32, 3]
    kp_wrp = kp_d.ap().rearrange("(s r) c -> r s c", r=16)         # [16, 256, 3]
    out_r = out_d.ap().rearrange("(cb g p) c -> cb p g c", g=GPC, p=128)

    AF = mybir.ActivationFunctionType
    OP = mybir.AluOpType

    with tile.TileContext(nc) as tc, contextlib.ExitStack() as ctx:
        lda = ctx.enter_context(tc.tile_pool(name="lda", bufs=4))
        tta = ctx.enter_context(tc.tile_pool(name="tta", bufs=4))
        meta = ctx.enter_context(tc.tile_pool(name="meta", bufs=1))
        gat = ctx.enter_context(tc.tile_pool(name="gat", bufs=3))
        blend = ctx.enter_context(tc.tile_pool(name="blend", bufs=3))
        dram = ctx.enter_context(tc.tile_pool(name="dram", bufs=1, space="DRAM"))
        TB = dram.tile([FLAT_PAD, 128], dt.float32)

        # ---- phase A: slab[c, px] -> TBEV[px, c] ----
        # StreamTranspose: TTB[32a+r, 32m+s] = BLK[32a+s, 32m+r]; one store
        # per 32-channel group a keeps both DMA access patterns at 3 dims.
        for kb in range(NLOAD):
            BLK = lda.tile([128, 2048], dt.float32, tag="BLK")
            nc.sync.dma_start(BLK[:], slab_d.ap()[:, kb * 2048:(kb + 1) * 2048])
            TTB = tta.tile([128, 2048], dt.float32, tag="TTB")
            nc.vector.transpose(TTB[:], BLK[:])
            for a in range(4):
                dst = TB[kb * 2048:(kb + 1) * 2048, a * 32:(a + 1) * 32] \
                    .rearrange("(m r) s -> r m s", r=32)
                src = TTB[a * 32:(a + 1) * 32, :].rearrange("p (m s) -> p m s", s=32)
                eng = (nc.sync, nc.scalar, nc.sync, nc.scalar)[a]
                eng.dma_start(dst, src)

        # ---- keypoint math ----
        def floor_of(v_ap, pool, nfree, tag):
            """floor(v) for v >= 0, exact under trunc or round f32<->i32."""
            CI = pool.tile([128, nfree], dt.int32, tag=tag + "i")
            nc.vector.tensor_copy(out=CI[:], in_=v_ap)
            CF = pool.tile([128, nfree], dt.float32, tag=tag + "f")
            nc.vector.tensor_copy(out=CF[:], in_=CI[:])
            GT = pool.tile([128, nfree], dt.float32, tag=tag + "g")
            nc.vector.tensor_tensor(GT[:], CF[:], v_ap, op=OP.is_gt)
            OUT = pool.tile([128, nfree], dt.float32, tag=tag + "o")
            nc.vector.tensor_tensor(OUT[:], CF[:], GT[:], op=OP.subtract)
            return OUT

        def coords(x_ap, y_ap, nfree, pfx):
            """-> (XS, YS, X0, Y0, Y1) f32 [128, nfree], reference clamps."""
            XS = meta.tile([128, nfree], dt.float32, tag=pfx + "XS")
            nc.scalar.activation(XS[:], x_ap, AF.Copy, bias=0.0, scale=scale)
            YS = meta.tile([128, nfree], dt.float32, tag=pfx + "YS")
            nc.scalar.activation(YS[:], y_ap, AF.Copy, bias=ybias, scale=scale)
            X0 = floor_of(XS[:], meta, nfree, pfx + "fx")
            T = floor_of(YS[:], meta, nfree, pfx + "fy")
            Y0 = meta.tile([128, nfree], dt.float32, tag=pfx + "Y0")
            nc.vector.tensor_scalar(Y0[:], T[:], float(H - 1), None, OP.min)
            Y1 = meta.tile([128, nfree], dt.float32, tag=pfx + "Y1")
            nc.vector.tensor_scalar(Y1[:], T[:], 1.0, float(H - 1), OP.add, OP.min)
            return XS, YS, X0, Y0, Y1

        # block-major pipeline: weights
        KP = meta.tile([128, 96], dt.float32)
        kp3 = KP[:].rearrange("p (s c) -> p s c", c=3)
        nc.sync.dma_start(kp3, kp_blk)
        XS, YS, X0, Y0, Y1 = coords(kp3[:, :, 0], kp3[:, :, 1], 32, "n")

        FX = meta.tile([128, 32], dt.float32)
        nc.vector.tensor_tensor(FX[:], XS[:], X0[:], op=OP.subtract)
        WXL = meta.tile([128, 32], dt.float32)
        nc.vector.tensor_scalar(WXL[:], FX[:], 1.0, -1.0, OP.subtract, OP.mult)
        WY0 = meta.tile([128, 32], dt.float32)
        nc.vector.tensor_tensor(WY0[:], Y1[:], YS[:], op=OP.subtract)
        WY1 = meta.tile([128, 32], dt.float32)
        nc.vector.tensor_tensor(WY1[:], YS[:], Y0[:], op=OP.subtract)
        # x parity qx = x0 mod 2 (row base y*178 is even)
        XH = meta.tile([128, 32], dt.float32)
        nc.vector.tensor_scalar(XH[:], X0[:], 0.5, None, OP.mult)
        XHF = floor_of(XH[:], meta, 32, "nqh")
        QX = meta.tile([128, 32], dt.float32)
        nc.vector.tensor_scalar(QX[:], XHF[:], -2.0, None, OP.mult)
        nc.vector.tensor_tensor(QX[:], X0[:], QX[:], op=OP.add)
        QM = meta.tile([128, 32], dt.float32)
        nc.vector.tensor_scalar(QM[:], QX[:], 1.0, -1.0, OP.subtract, OP.mult)
        # 3-slot x weights: u0 = wxl*(1-qx); u1 = wxl*qx + fx*(1-qx); u2 = fx*qx
        U0 = meta.tile([128, 32], dt.float32)
        nc.vector.tensor_tensor(U0[:], WXL[:], QM[:], op=OP.mult)
        U1 = meta.tile([128, 32], dt.float32)
        T1 = meta.tile([128, 32], dt.float32)
        nc.vector.tensor_tensor(T1[:], WXL[:], QX[:], op=OP.mult)
        nc.vector.tensor_tensor(U1[:], FX[:], QM[:], op=OP.mult)
        nc.vector.tensor_tensor(U1[:], U1[:], T1[:], op=OP.add)
        U2 = meta.tile([128, 32], dt.float32)
        nc.vector.tensor_tensor(U2[:], FX[:], QX[:], op=OP.mult)
        W3AC = meta.tile([128, 32, 3], dt.float32)
        W3BD = meta.tile([128, 32, 3], dt.float32)
        for k, u in enumerate((U0, U1, U2)):
            nc.vector.tensor_tensor(W3AC[:, :, k], u[:], WY0[:], op=OP.mult)
            nc.vector.tensor_tensor(W3BD[:, :, k], u[:], WY1[:], op=OP.mult)

        # wrapped-16 pipeline: gather pair-row indices (int16)
        KPW = meta.tile([128, 768], dt.float32)
        kpw3 = KPW[:].rearrange("p (s c) -> p s c", c=3)
        for g in range(8):
            nc.sync.dma_start(kpw3[g * 16:(g + 1) * 16], kp_wrp)
        _, _, X0w, Y0w, Y1w = coords(kpw3[:, :, 0], kpw3[:, :, 1], 256, "w")
        IDXW = []
        for nm, yy in (("I0", Y0w), ("I1", Y1w)):
            base = meta.tile([128, 256], dt.float32, tag=nm + "b")
            nc.vector.tensor_scalar(base[:], yy[:], float(W_PACK), None, OP.mult)
            nc.vector.tensor_tensor(base[:], base[:], X0w[:], op=OP.add)
            nc.vector.tensor_scalar(base[:], base[:], 0.5, None, OP.mult)
            bf = floor_of(base[:], meta, 256, nm + "fh")
            ii = meta.tile([128, 256], dt.int16, tag=nm + "w")
            nc.vector.tensor_copy(out=ii[:], in_=bf[:])
            IDXW.append(ii)
        I0W, I1W = IDXW

        if debug_taps:
            nc.sync.dma_start(taps["t_X0"].ap(), X0[:])
            nc.sync.dma_start(taps["t_QX"].ap(), QX[:])
            nc.sync.dma_start(taps["t_W3AC"].ap(),
                              W3AC[:].rearrange("p s c -> p (s c)"))
            I0f = meta.tile([128, 256], dt.float32)
            nc.vector.tensor_copy(out=I0f[:], in_=I0W[:])
            nc.sync.dma_start(taps["t_I0w"].ap(), I0f[:])
            nc.sync.dma_start(taps["t_TB"].ap(), TB[0:128, :])

        # overlapped pair-row view of TBEV: row v = elements [v*256, v*256+384)
        tb_pairs = AP(TB[:].tensor, TB[:].offset, [[256, VPAIR], [1, 384]])

        # ---- phase B: gather + in-place blend + store ----
        for cb in range(NBCH):
            wsl = slice(cb * (BCH // 16), (cb + 1) * (BCH // 16))
            bsl = slice(cb * GPC, (cb + 1) * GPC)
            G0 = gat.tile([128, GPC, 3, 128], dt.float32, tag="G0")
            nc.gpsimd.dma_gather(
                out_ap=G0[:].rearrange("p g t c -> p g (t c)"),
                in_ap=tb_pairs, idxs_ap=I0W[:, wsl],
                num_idxs=BCH, num_idxs_reg=BCH, elem_size=384, elem_step=256)
            G1 = gat.tile([128, GPC, 3, 128], dt.float32, tag="G1")
            nc.gpsimd.dma_gather(
                out_ap=G1[:].rearrange("p g t c -> p g (t c)"),
                in_ap=tb_pairs, idxs_ap=I1W[:, wsl],
                num_idxs=BCH, num_idxs_reg=BCH, elem_size=384, elem_step=256)

            P0 = blend.tile([128, GPC, 3, 128], dt.float32, tag="P0")
            w3ac_b = W3AC[:, bsl, :, None].to_broadcast((128, GPC, 3, 128))
            nc.vector.tensor_tensor(P0[:], G0[:], w3ac_b, op=OP.mult)
            P1 = blend.tile([128, GPC, 3, 128], dt.float32, tag="P1")
            w3bd_b = W3BD[:, bsl, :, None].to_broadcast((128, GPC, 3, 128))
            nc.vector.tensor_tensor(P1[:], G1[:], w3bd_b, op=OP.mult)
            S = blend.tile([128, GPC, 3, 128], dt.float32, tag="S")
            nc.vector.tensor_tensor(S[:], P0[:], P1[:], op=OP.add)
            OUTG = blend.tile([128, GPC, 128], dt.float32, tag="OUTG")
            nc.vector.tensor_tensor(OUTG[:], S[:, :, 0, :], S[:, :, 1, :], op=OP.add)
            nc.vector.tensor_tensor(OUTG[:], OUTG[:], S[:, :, 2, :], op=OP.add)
            nc.sync.dma_start(out_r[cb], OUTG[:])

    nc.compile()
    return nc


def _get(scale: float, ybias: float):
    key = (round(scale, 9), round(ybias, 9))
    if key not in _compiled:
        _compiled[key] = _build(scale, ybias)
    return _compiled[key]


def _prepare_in_maps(keypoints: np.ndarray, bev_features: np.ndarray):
    kp = np.ascontiguousarray(keypoints, dtype=np.float32)
    bev = np.asarray(bev_features, dtype=np.float32)
    in_maps = []
    for core in range(N_CORES):
        b, ch = core // 2, core % 2
        sl = slice(ch * 128, (ch + 1) * 128)
        slab = np.zeros((128, FLAT_PAD), dtype=np.float32)
        slab[:, :FLAT] = bev[b, sl, :, :W_PACK].reshape(128, FLAT)
        in_maps.append({"slab": slab, "kp": kp[b]})
    return in_maps


def _assemble(results) -> np.ndarray:
    out = np.empty((B, N, C), dtype=np.float32)
    for core in range(N_CORES):
        b, ch = core // 2, core % 2
        out[b, :, ch * 128:(ch + 1) * 128] = np.asarray(results[core]["out"])
    return out


def _scale_bias(bev_stride):
    stride = float(np.asarray(bev_stride))
    scale = 1.0 / (0.05 * stride)
    return scale, 40.0 * scale


def kernel(keypoints: np.ndarray, bev_features: np.ndarray, bev_stride) -> np.ndarray:
    from concourse.bass_utils import run_bass_kernel_spmd

    scale, ybias = _scale_bias(bev_stride)
    nc = _get(scale, ybias)
    in_maps = _prepare_in_maps(keypoints, bev_features)
    res = run_bass_kernel_spmd(nc, in_maps, list(range(N_CORES))).results
    return _assemble(res)
